# revision 7
# baseline (speedup 1.0000x reference)
"""Additive (Bahdanau) attention on 8 TRN2 NeuronCores, data-parallel over batch.

Reference computation (per batch b):
  q = query @ Wq                    [Q, H]
  k = key @ Wk                      [K, H]
  scores[q,k] = sum_h Wv[h] * tanh(q[q,h] + k[k,h])
  masked softmax over k (k >= valid_len[b] -> -1e6), out = attn @ value

Device strategy per core (2 batches/core):
  - Host pre-transposes query/key to [D, seq] layout and precomputes the
    mask tile; Wv reshaped to [H, 1].
  - PE: qT = Wq^T @ queryT, kT = Wk^T @ keyT  -> [H=128 part, seq] (fp32)
  - DVE: s[:, q*K:(q+1)*K] = kT + qT[:, q]  via tensor_scalar_add
    (per-partition scalar = broadcast add), bf16
  - ACT: t = tanh(s) in big chunks (the throughput floor of the kernel)
  - PE: scoresT columns: matmul(lhsT=t[h, 128k-block], rhs=Wv[h,1]) -> [128k, 1]
    written at free-offset q of a persistent PSUM tile [128k, (kc, q)]
  - PE transpose -> scores [q-part, k-free]; DVE/ACT masked softmax
    (min with mask, -max, exp with accum_out row sums, reciprocal, scale)
  - PE transpose attn -> attnT; AV matmul attnT^T... out[q, dv]; DMA out.
"""

import sys
import numpy as np

if "/opt/trn_rl_repo" not in sys.path:
    sys.path.insert(0, "/opt/trn_rl_repo")

B, Q, K, DQ, DK, H, DV = 16, 256, 256, 256, 256, 128, 256
NCORES = 8
BPC = B // NCORES  # batches per core
NEG = -1e6
QCHUNK = 32  # q rows per tanh chunk

_cache = {}


def _build_nc():
    from contextlib import ExitStack

    from concourse import bacc, mybir, tile
    from concourse.masks import make_identity

    f32 = mybir.dt.float32
    bf16 = mybir.dt.bfloat16
    AF = mybir.ActivationFunctionType
    ALU = mybir.AluOpType
    AX = mybir.AxisListType

    nc = bacc.Bacc(
        "TRN2",
        target_bir_lowering=False,
        debug=False,
        enable_asserts=False,
        num_devices=NCORES,
    )

    d_qT = nc.dram_tensor("queryT", [BPC, DQ, Q], f32, kind="ExternalInput")
    d_kT = nc.dram_tensor("keyT", [BPC, DK, K], f32, kind="ExternalInput")
    d_v = nc.dram_tensor("value", [BPC, K, DV], f32, kind="ExternalInput")
    d_wq = nc.dram_tensor("Wq", [DQ, H], f32, kind="ExternalInput")
    d_wk = nc.dram_tensor("Wk", [DK, H], f32, kind="ExternalInput")
    d_wv = nc.dram_tensor("Wv", [H, 1], f32, kind="ExternalInput")
    d_mask = nc.dram_tensor("mask", [BPC, 128, K], f32, kind="ExternalInput")
    d_out = nc.dram_tensor("out", [BPC, Q, DV], f32, kind="ExternalOutput")

    with tile.TileContext(nc) as tc, ExitStack() as ctx:
        const_p = ctx.enter_context(tc.tile_pool(name="const", bufs=1))
        io_p = ctx.enter_context(tc.tile_pool(name="io", bufs=2))
        work_p = ctx.enter_context(tc.tile_pool(name="work", bufs=2))
        sm_p = ctx.enter_context(tc.tile_pool(name="sm", bufs=2))
        ps_proj = ctx.enter_context(tc.tile_pool(name="ps_proj", bufs=1, space="PSUM"))
        ps_scT = ctx.enter_context(tc.tile_pool(name="ps_scT", bufs=2, space="PSUM"))
        ps_sc = ctx.enter_context(tc.tile_pool(name="ps_sc", bufs=2, space="PSUM"))
        ps_at = ctx.enter_context(tc.tile_pool(name="ps_at", bufs=1, space="PSUM"))
        ps_av = ctx.enter_context(tc.tile_pool(name="ps_av", bufs=1, space="PSUM"))

        ident_f = const_p.tile([128, 128], f32)
        make_identity(nc, ident_f)
        ident_b = const_p.tile([128, 128], bf16)
        make_identity(nc, ident_b)

        wv_sb = const_p.tile([128, 1], bf16)
        nc.gpsimd.dma_start(out=wv_sb, in_=d_wv.ap())
        # Wq/Wk as [128 part (dq in chunk), (chunk, h)]
        wq_sb = const_p.tile([128, 2, H], f32)
        nc.sync.dma_start(out=wq_sb, in_=d_wq.ap().rearrange("(c p) h -> p c h", p=128))
        wk_sb = const_p.tile([128, 2, H], f32)
        nc.sync.dma_start(out=wk_sb, in_=d_wk.ap().rearrange("(c p) h -> p c h", p=128))

        for b in range(BPC):
            # ---- projections: qT/kT [H=128, seq] ----
            qT_ps = ps_proj.tile([128, Q], f32, tag="qT_ps")
            kT_ps = ps_proj.tile([128, K], f32, tag="kT_ps")
            for c in range(2):
                qts = io_p.tile([128, Q], f32, tag="qts")
                nc.sync.dma_start(out=qts, in_=d_qT.ap()[b, c * 128 : (c + 1) * 128, :])
                nc.tensor.matmul(
                    out=qT_ps, lhsT=wq_sb[:, c, :], rhs=qts,
                    start=(c == 0), stop=(c == 1),
                )
                kts = io_p.tile([128, K], f32, tag="kts")
                nc.sync.dma_start(out=kts, in_=d_kT.ap()[b, c * 128 : (c + 1) * 128, :])
                nc.tensor.matmul(
                    out=kT_ps, lhsT=wk_sb[:, c, :], rhs=kts,
                    start=(c == 0), stop=(c == 1),
                )
            qT_sb = io_p.tile([128, Q], f32, tag="qT_sb")
            nc.vector.tensor_copy(out=qT_sb, in_=qT_ps)
            kT_bf = io_p.tile([128, K], bf16, tag="kT_bf")
            nc.vector.tensor_copy(out=kT_bf, in_=kT_ps)

            # value (cast to bf16): [128 part(k in chunk), (kc, dv)]
            val_bf = io_p.tile([128, 2, DV], bf16, tag="val_bf")
            nc.gpsimd.dma_start(
                out=val_bf, in_=d_v.ap()[b].rearrange("(c p) dv -> p c dv", p=128)
            )
            mask_sb = io_p.tile([128, K], f32, tag="mask_sb")
            nc.sync.dma_start(out=mask_sb, in_=d_mask.ap()[b])

            # ---- main: tanh + scores ----
            # scoresT psum tile: [128 part (k within chunk), (kc, q)] fp32
            scT_ps = ps_scT.tile([128, 2, Q], f32)
            for ch in range(Q // QCHUNK):
                s_chunk = work_p.tile([128, QCHUNK * K], bf16, tag="s_chunk")
                for qi in range(QCHUNK):
                    qg = ch * QCHUNK + qi
                    nc.vector.tensor_scalar_add(
                        out=s_chunk[:, qi * K : (qi + 1) * K],
                        in0=kT_bf,
                        scalar1=qT_sb[:, qg : qg + 1],
                    )
                t_chunk = work_p.tile([128, QCHUNK * K], bf16, tag="t_chunk")
                nc.scalar.activation(out=t_chunk, in_=s_chunk, func=AF.Tanh)
                for qi in range(QCHUNK):
                    qg = ch * QCHUNK + qi
                    for kc in range(2):
                        nc.tensor.matmul(
                            out=scT_ps[:, kc, qg : qg + 1],
                            lhsT=t_chunk[:, qi * K + kc * 128 : qi * K + (kc + 1) * 128],
                            rhs=wv_sb,
                            start=True, stop=True,
                        )

            # ---- scoresT -> scores, softmax, AV ----
            scT_sb = sm_p.tile([128, 2, Q], f32, tag="scT_sb")
            nc.vector.tensor_copy(out=scT_sb, in_=scT_ps)
            for qh in range(2):
                sc_ps = ps_sc.tile([128, K], f32, tag="sc_ps")
                for kc in range(2):
                    nc.tensor.transpose(
                        out=sc_ps[:, kc * 128 : (kc + 1) * 128],
                        in_=scT_sb[:, kc, qh * 128 : (qh + 1) * 128],
                        identity=ident_f,
                    )
                sc_sb = sm_p.tile([128, K], f32, tag="sc_sb")
                nc.vector.tensor_tensor(
                    out=sc_sb, in0=sc_ps, in1=mask_sb, op=ALU.min
                )
                negmax = sm_p.tile([128, 1], f32, tag="negmax")
                nc.vector.tensor_reduce(
                    out=negmax, in_=sc_sb, axis=AX.X, op=ALU.max, negate=True
                )
                p_bf = sm_p.tile([128, K], bf16, tag="p_bf")
                rowsum = sm_p.tile([128, 1], f32, tag="rowsum")
                nc.scalar.activation(
                    out=p_bf, in_=sc_sb, func=AF.Exp,
                    bias=negmax, scale=1.0, accum_out=rowsum,
                )
                rinv = sm_p.tile([128, 1], f32, tag="rinv")
                nc.vector.reciprocal(out=rinv, in_=rowsum)
                attn_bf = sm_p.tile([128, K], bf16, tag="attn_bf")
                nc.vector.tensor_scalar_mul(out=attn_bf, in0=p_bf, scalar1=rinv)

                attnT_ps = ps_at.tile([128, 2, 128], bf16, tag="attnT_ps")
                for kc in range(2):
                    nc.tensor.transpose(
                        out=attnT_ps[:, kc, :],
                        in_=attn_bf[:, kc * 128 : (kc + 1) * 128],
                        identity=ident_b,
                    )
                attnT_sb = sm_p.tile([128, 2, 128], bf16, tag="attnT_sb")
                nc.vector.tensor_copy(out=attnT_sb, in_=attnT_ps)

                av_ps = ps_av.tile([128, DV], f32, tag="av_ps")
                for kc in range(2):
                    nc.tensor.matmul(
                        out=av_ps,
                        lhsT=attnT_sb[:, kc, :],
                        rhs=val_bf[:, kc, :],
                        start=(kc == 0), stop=(kc == 1),
                    )
                out_sb = sm_p.tile([128, DV], f32, tag="out_sb")
                nc.vector.tensor_copy(out=out_sb, in_=av_ps)
                nc.sync.dma_start(
                    out=d_out.ap()[b, qh * 128 : (qh + 1) * 128, :], in_=out_sb
                )

    nc.compile()
    return nc


def _get_nc():
    if "nc" not in _cache:
        _cache["nc"] = _build_nc()
    return _cache["nc"]


def _make_in_maps(query, key, value, Wq, Wk, Wv, valid_len):
    query = np.asarray(query, dtype=np.float32)
    key = np.asarray(key, dtype=np.float32)
    value = np.asarray(value, dtype=np.float32)
    Wq = np.ascontiguousarray(np.asarray(Wq, dtype=np.float32))
    Wk = np.ascontiguousarray(np.asarray(Wk, dtype=np.float32))
    Wv = np.ascontiguousarray(np.asarray(Wv, dtype=np.float32).reshape(H, 1))
    vl = np.asarray(valid_len).astype(np.int64)

    queryT = np.ascontiguousarray(query.transpose(0, 2, 1))  # [B, DQ, Q]
    keyT = np.ascontiguousarray(key.transpose(0, 2, 1))  # [B, DK, K]
    # mask tile: min(scores, mask) -> +big keeps, NEG masks (exactly as ref)
    kidx = np.arange(K)[None, :]
    mrow = np.where(kidx < vl[:, None], np.float32(1e9), np.float32(NEG))
    mask = np.ascontiguousarray(
        np.broadcast_to(mrow[:, None, :], (B, 128, K)).astype(np.float32)
    )

    in_maps = []
    for c in range(NCORES):
        sl = slice(c * BPC, (c + 1) * BPC)
        in_maps.append(
            {
                "queryT": queryT[sl],
                "keyT": keyT[sl],
                "value": np.ascontiguousarray(value[sl]),
                "Wq": Wq,
                "Wk": Wk,
                "Wv": Wv,
                "mask": mask[sl],
            }
        )
    return in_maps


def kernel(query, key, value, Wq, Wk, Wv, valid_len):
    from concourse import bass_utils

    nc = _get_nc()
    in_maps = _make_in_maps(query, key, value, Wq, Wk, Wv, valid_len)
    res = bass_utils.run_bass_kernel_spmd(nc, in_maps, core_ids=list(range(NCORES)))
    out = np.concatenate([np.asarray(r["out"]) for r in res.results], axis=0)
    return out.astype(np.float32)


# revision 9
# speedup vs baseline: 7.9261x; 7.9261x over previous
"""Additive (Bahdanau) attention on 8 TRN2 NeuronCores, data-parallel over batch.

Reference computation (per batch b):
  q = query @ Wq                    [Q, H]
  k = key @ Wk                      [K, H]
  scores[q,k] = sum_h Wv[h] * tanh(q[q,h] + k[k,h])
  masked softmax over k (k >= valid_len[b] -> -1e6), out = attn @ value

Device strategy per core (2 batches/core):
  - Host pre-transposes query/key to [D, seq] layout and precomputes the
    mask tile; Wv reshaped to [H, 1].
  - PE: qT = Wq^T @ queryT, kT = Wk^T @ keyT  -> [H=128 part, seq] (fp32)
  - DVE: s[:, q*K:(q+1)*K] = kT + qT[:, q]  via tensor_scalar_add
    (per-partition scalar = broadcast add), bf16
  - ACT: t = tanh(s) in big chunks (the throughput floor of the kernel)
  - PE: scoresT columns: matmul(lhsT=t[h, 128k-block], rhs=Wv[h,1]) -> [128k, 1]
    written at free-offset q of a persistent PSUM tile [128k, (kc, q)]
  - PE transpose -> scores [q-part, k-free]; DVE/ACT masked softmax
    (min with mask, -max, exp with accum_out row sums, reciprocal, scale)
  - PE transpose attn -> attnT; AV matmul attnT^T... out[q, dv]; DMA out.
"""

import sys
import numpy as np

if "/opt/trn_rl_repo" not in sys.path:
    sys.path.insert(0, "/opt/trn_rl_repo")

B, Q, K, DQ, DK, H, DV = 16, 256, 256, 256, 256, 128, 256
NCORES = 8
BPC = B // NCORES  # batches per core
NEG = -1e6
QCHUNK = 32  # q rows per tanh chunk

_cache = {}


def _build_nc(repeat=1):
    from contextlib import ExitStack

    from concourse import bacc, mybir, tile
    from concourse.masks import make_identity

    f32 = mybir.dt.float32
    bf16 = mybir.dt.bfloat16
    AF = mybir.ActivationFunctionType
    ALU = mybir.AluOpType
    AX = mybir.AxisListType

    nc = bacc.Bacc(
        "TRN2",
        target_bir_lowering=False,
        debug=False,
        enable_asserts=False,
        num_devices=NCORES,
    )

    d_qT = nc.dram_tensor("queryT", [BPC, DQ, Q], f32, kind="ExternalInput")
    d_kT = nc.dram_tensor("keyT", [BPC, DK, K], f32, kind="ExternalInput")
    d_v = nc.dram_tensor("value", [BPC, K, DV], f32, kind="ExternalInput")
    d_wq = nc.dram_tensor("Wq", [DQ, H], f32, kind="ExternalInput")
    d_wk = nc.dram_tensor("Wk", [DK, H], f32, kind="ExternalInput")
    d_wv = nc.dram_tensor("Wv", [H, 1], f32, kind="ExternalInput")
    d_mask = nc.dram_tensor("mask", [BPC, 128, K], f32, kind="ExternalInput")
    d_out = nc.dram_tensor("out", [BPC, Q, DV], f32, kind="ExternalOutput")

    with tile.TileContext(nc) as tc, ExitStack() as ctx:
        const_p = ctx.enter_context(tc.tile_pool(name="const", bufs=1))
        io_p = ctx.enter_context(tc.tile_pool(name="io", bufs=2))
        work_p = ctx.enter_context(tc.tile_pool(name="work", bufs=2))
        sm_p = ctx.enter_context(tc.tile_pool(name="sm", bufs=2))
        ps_proj = ctx.enter_context(tc.tile_pool(name="ps_proj", bufs=1, space="PSUM"))
        ps_scT = ctx.enter_context(tc.tile_pool(name="ps_scT", bufs=2, space="PSUM"))
        ps_sc = ctx.enter_context(tc.tile_pool(name="ps_sc", bufs=2, space="PSUM"))
        ps_at = ctx.enter_context(tc.tile_pool(name="ps_at", bufs=1, space="PSUM"))
        ps_av = ctx.enter_context(tc.tile_pool(name="ps_av", bufs=1, space="PSUM"))

        ident_f = const_p.tile([128, 128], f32)
        make_identity(nc, ident_f)
        ident_b = const_p.tile([128, 128], bf16)
        make_identity(nc, ident_b)

        wv_sb = const_p.tile([128, 1], bf16)
        nc.gpsimd.dma_start(out=wv_sb, in_=d_wv.ap())
        # Wq/Wk as [128 part (dq in chunk), (chunk, h)]
        wq_sb = const_p.tile([128, 2, H], f32)
        nc.sync.dma_start(out=wq_sb, in_=d_wq.ap().rearrange("(c p) h -> p c h", p=128))
        wk_sb = const_p.tile([128, 2, H], f32)
        nc.sync.dma_start(out=wk_sb, in_=d_wk.ap().rearrange("(c p) h -> p c h", p=128))

        for b in [bb % BPC for bb in range(BPC * repeat)]:
            # ---- projections: qT/kT [H=128, seq] ----
            qT_ps = ps_proj.tile([128, Q], f32, tag="qT_ps")
            kT_ps = ps_proj.tile([128, K], f32, tag="kT_ps")
            for c in range(2):
                qts = io_p.tile([128, Q], f32, tag="qts")
                nc.sync.dma_start(out=qts, in_=d_qT.ap()[b, c * 128 : (c + 1) * 128, :])
                nc.tensor.matmul(
                    out=qT_ps, lhsT=wq_sb[:, c, :], rhs=qts,
                    start=(c == 0), stop=(c == 1),
                )
                kts = io_p.tile([128, K], f32, tag="kts")
                nc.sync.dma_start(out=kts, in_=d_kT.ap()[b, c * 128 : (c + 1) * 128, :])
                nc.tensor.matmul(
                    out=kT_ps, lhsT=wk_sb[:, c, :], rhs=kts,
                    start=(c == 0), stop=(c == 1),
                )
            qT_sb = io_p.tile([128, Q], f32, tag="qT_sb")
            nc.vector.tensor_copy(out=qT_sb, in_=qT_ps)
            kT_bf = io_p.tile([128, K], bf16, tag="kT_bf")
            nc.vector.tensor_copy(out=kT_bf, in_=kT_ps)

            # value (cast to bf16): [128 part(k in chunk), (kc, dv)]
            val_bf = io_p.tile([128, 2, DV], bf16, tag="val_bf")
            nc.gpsimd.dma_start(
                out=val_bf, in_=d_v.ap()[b].rearrange("(c p) dv -> p c dv", p=128)
            )
            mask_sb = io_p.tile([128, K], f32, tag="mask_sb")
            nc.sync.dma_start(out=mask_sb, in_=d_mask.ap()[b])

            # ---- main: tanh + scores ----
            # scoresT psum tile: [128 part (k within chunk), (kc, q)] fp32
            scT_ps = ps_scT.tile([128, 2, Q], f32)
            for ch in range(Q // QCHUNK):
                s_chunk = work_p.tile([128, QCHUNK * K], bf16, tag="s_chunk")
                for qi in range(QCHUNK):
                    qg = ch * QCHUNK + qi
                    nc.vector.tensor_scalar_add(
                        out=s_chunk[:, qi * K : (qi + 1) * K],
                        in0=kT_bf,
                        scalar1=qT_sb[:, qg : qg + 1],
                    )
                t_chunk = work_p.tile([128, QCHUNK * K], bf16, tag="t_chunk")
                nc.scalar.activation(out=t_chunk, in_=s_chunk, func=AF.Tanh)
                for qi in range(QCHUNK):
                    qg = ch * QCHUNK + qi
                    for kc in range(2):
                        nc.tensor.matmul(
                            out=scT_ps[:, kc, qg : qg + 1],
                            lhsT=t_chunk[:, qi * K + kc * 128 : qi * K + (kc + 1) * 128],
                            rhs=wv_sb,
                            start=True, stop=True,
                        )

            # ---- scoresT -> scores, softmax, AV ----
            scT_sb = sm_p.tile([128, 2, Q], f32, tag="scT_sb")
            nc.vector.tensor_copy(out=scT_sb, in_=scT_ps)
            for qh in range(2):
                sc_ps = ps_sc.tile([128, K], f32, tag="sc_ps")
                for kc in range(2):
                    nc.tensor.transpose(
                        out=sc_ps[:, kc * 128 : (kc + 1) * 128],
                        in_=scT_sb[:, kc, qh * 128 : (qh + 1) * 128],
                        identity=ident_f,
                    )
                sc_sb = sm_p.tile([128, K], f32, tag="sc_sb")
                nc.vector.tensor_tensor(
                    out=sc_sb, in0=sc_ps, in1=mask_sb, op=ALU.min
                )
                negmax = sm_p.tile([128, 1], f32, tag="negmax")
                nc.vector.tensor_reduce(
                    out=negmax, in_=sc_sb, axis=AX.X, op=ALU.max, negate=True
                )
                p_bf = sm_p.tile([128, K], bf16, tag="p_bf")
                rowsum = sm_p.tile([128, 1], f32, tag="rowsum")
                nc.scalar.activation(
                    out=p_bf, in_=sc_sb, func=AF.Exp,
                    bias=negmax, scale=1.0, accum_out=rowsum,
                )
                rinv = sm_p.tile([128, 1], f32, tag="rinv")
                nc.vector.reciprocal(out=rinv, in_=rowsum)
                attn_bf = sm_p.tile([128, K], bf16, tag="attn_bf")
                nc.vector.tensor_scalar_mul(out=attn_bf, in0=p_bf, scalar1=rinv)

                attnT_ps = ps_at.tile([128, 2, 128], bf16, tag="attnT_ps")
                for kc in range(2):
                    nc.tensor.transpose(
                        out=attnT_ps[:, kc, :],
                        in_=attn_bf[:, kc * 128 : (kc + 1) * 128],
                        identity=ident_b,
                    )
                attnT_sb = sm_p.tile([128, 2, 128], bf16, tag="attnT_sb")
                nc.vector.tensor_copy(out=attnT_sb, in_=attnT_ps)

                av_ps = ps_av.tile([128, DV], f32, tag="av_ps")
                for kc in range(2):
                    nc.tensor.matmul(
                        out=av_ps,
                        lhsT=attnT_sb[:, kc, :],
                        rhs=val_bf[:, kc, :],
                        start=(kc == 0), stop=(kc == 1),
                    )
                out_sb = sm_p.tile([128, DV], f32, tag="out_sb")
                nc.vector.tensor_copy(out=out_sb, in_=av_ps)
                nc.sync.dma_start(
                    out=d_out.ap()[b, qh * 128 : (qh + 1) * 128, :], in_=out_sb
                )

    nc.compile()
    return nc


def _get_nc():
    if "nc" not in _cache:
        _cache["nc"] = _build_nc()
    return _cache["nc"]


def _make_in_maps(query, key, value, Wq, Wk, Wv, valid_len):
    query = np.asarray(query, dtype=np.float32)
    key = np.asarray(key, dtype=np.float32)
    value = np.asarray(value, dtype=np.float32)
    Wq = np.ascontiguousarray(np.asarray(Wq, dtype=np.float32))
    Wk = np.ascontiguousarray(np.asarray(Wk, dtype=np.float32))
    Wv = np.ascontiguousarray(np.asarray(Wv, dtype=np.float32).reshape(H, 1))
    vl = np.asarray(valid_len).astype(np.int64)

    queryT = np.ascontiguousarray(query.transpose(0, 2, 1))  # [B, DQ, Q]
    keyT = np.ascontiguousarray(key.transpose(0, 2, 1))  # [B, DK, K]
    # mask tile: min(scores, mask) -> +big keeps, NEG masks (exactly as ref)
    kidx = np.arange(K)[None, :]
    mrow = np.where(kidx < vl[:, None], np.float32(1e9), np.float32(NEG))
    mask = np.ascontiguousarray(
        np.broadcast_to(mrow[:, None, :], (B, 128, K)).astype(np.float32)
    )

    in_maps = []
    for c in range(NCORES):
        sl = slice(c * BPC, (c + 1) * BPC)
        in_maps.append(
            {
                "queryT": queryT[sl],
                "keyT": keyT[sl],
                "value": np.ascontiguousarray(value[sl]),
                "Wq": Wq,
                "Wk": Wk,
                "Wv": Wv,
                "mask": mask[sl],
            }
        )
    return in_maps


def kernel(query, key, value, Wq, Wk, Wv, valid_len):
    from concourse import bass_utils

    nc = _get_nc()
    in_maps = _make_in_maps(query, key, value, Wq, Wk, Wv, valid_len)
    res = bass_utils.run_bass_kernel_spmd(nc, in_maps, core_ids=list(range(NCORES)))
    out = np.concatenate([np.asarray(r["out"]) for r in res.results], axis=0)
    return out.astype(np.float32)
